# revision 5
# baseline (speedup 1.0000x reference)
"""Trainium2 Bass kernel for spatial multi-head self-attention
(conv1x1 qkv -> 4-head attention over n=4096 tokens -> conv1x1 out + residual).

Sharding: 8 cores = 2 batches x 4 heads; each core runs one (batch, head)
attention and emits the UN-normalized head context [V^T|1]P (33 rows: 32 dims
+ softmax denominator) for its head; the host divides by the denominator,
applies the output projection, sums the 4 head partials per batch and adds
bias + residual (small numpy epilogue).

Device pipeline per core (all matmuls bf16/f32r, fp32 accumulate):
  prep: x DMA'd f32 and used directly as f32r (bitcast, no convert);
        k4/q4 = head K/Q replicated at partition blocks 0/32/64/96 so sim
        matmuls row-pack 4x (K=32, concurrent row-tiles); vT1[j] = [V^T|1].
  per i-tile (512 tokens): 9 sim groups of 4/3 j-tiles (row-packed, PSUM
        4-bank/3-bank alternating) -> exp SPLIT across two engines:
        ACT does native Exp on the 4-groups, DVE does Schraudolph exp
        (tensor_scalar f32*A+C -> int16 -> bitcast bf16) on the 3-groups --
        the exp wall is the kernel's roofline and the split nearly doubles
        exp throughput; AV matmuls are 2x column-tiled (even j-tiles ->
        PSUM partitions 0:33, odd -> 64:97, concurrent col groups).
  epilogue per i-tile: one DVE copy PSUM->SBUF + DMA out (no on-device
        normalize/projection).
"""

import numpy as np

B, C, H, W = 2, 128, 64, 64
N = H * W            # 4096
HEADS = 4
DH = 32              # head dim
NT = 512             # i-tile width
NIT = N // NT        # 8 i-tiles
JT = 128             # j-tile width
NJT = N // JT        # 32 j-tiles
GROUPS = [4, 3, 4, 3, 4, 3, 4, 3, 4]          # j-tiles per sim/exp group
EXP_DVE = [False, True, False, True, False, True, False, True, False]
SCALE = DH ** -0.5
EXPA = 128.0 / np.log(2.0)   # Schraudolph bf16: bits = rint(s*EXPA + EXPC)
EXPC = 16248.6

_CACHE = {}


def _build():
    if "nc" in _CACHE:
        return _CACHE["nc"]

    import concourse.bacc as bacc
    import concourse.mybir as mybir
    import concourse.tile as tile

    F32 = mybir.dt.float32
    F32R = mybir.dt.float32r
    BF16 = mybir.dt.bfloat16
    I16 = mybir.dt.int16
    AF = mybir.ActivationFunctionType
    MULT = mybir.AluOpType.mult
    ADD = mybir.AluOpType.add

    nc = bacc.Bacc("TRN2", target_bir_lowering=False, debug=False, num_devices=8)

    xt = nc.dram_tensor("xt", [C, N], F32R, kind="ExternalInput")
    wq4 = nc.dram_tensor("wq4", [C, 128], F32R, kind="ExternalInput")
    wk4 = nc.dram_tensor("wk4", [C, 128], F32R, kind="ExternalInput")
    wv = nc.dram_tensor("wv", [C, DH], F32R, kind="ExternalInput")
    o_out = nc.dram_tensor("o_out", [98, N], F32, kind="ExternalOutput")

    with tile.TileContext(nc) as tc:
        with (
            tc.tile_pool(name="const", bufs=1) as cp,
            tc.tile_pool(name="work", bufs=2) as wp,
            tc.tile_pool(name="work3", bufs=3) as wp3,
            tc.tile_pool(name="ps_sim4", bufs=1, space="PSUM") as ps4,
            tc.tile_pool(name="ps_sim3", bufs=1, space="PSUM") as ps3,
            tc.tile_pool(name="ps_o", bufs=1, space="PSUM") as ps_o,
        ):
            # ---- weights (f32 staged, viewed as f32r for the PE) ----
            wq_sb = cp.tile([C, 128], F32R, tag="wq_sb")
            nc.sync.dma_start(wq_sb[:], wq4.ap())
            wk_sb = cp.tile([C, 128], F32R, tag="wk_sb")
            nc.sync.dma_start(wk_sb[:], wk4.ap())
            wv_sb = cp.tile([C, DH], F32R, tag="wv_sb")
            nc.sync.dma_start(wv_sb[:], wv.ap())

            # ---- x: chunked DMA, no conversion (f32r view) ----
            x_sb = cp.tile([C, N], F32R, tag="x_sb")
            for ci in range(NIT):
                s = slice(ci * NT, (ci + 1) * NT)
                nc.sync.dma_start(x_sb[:, s], xt.ap()[:, s])

            # ---- k4 / q4 projections (replica-packed stationary) ----
            # chunks of 3/3/2 through the 4-bank / 3-bank sim pools so the
            # evacuations pipeline; evac alternates ACT/DVE.
            k4 = cp.tile([128, N], BF16, tag="k4")
            q4 = cp.tile([128, N], BF16, tag="q4")
            for dst, wsb in ((k4, wk_sb), (q4, wq_sb)):
                for gi, (c0, gc) in enumerate(((0, 3), (3, 3), (6, 2))):
                    pp = ps4 if gi % 2 == 0 else ps3
                    kp = pp.tile([128, 2048 if gi % 2 == 0 else 1536], F32,
                                 tag="s4" if gi % 2 == 0 else "s3")
                    for m in range(gc):
                        ch = c0 + m
                        nc.tensor.matmul(
                            kp[:, NT * m:NT * (m + 1)],
                            wsb[:],
                            x_sb[:, ch * NT:(ch + 1) * NT],
                            start=True, stop=True)
                    sl = slice(c0 * NT, (c0 + gc) * NT)
                    if gi % 2 == 0:
                        nc.scalar.copy(dst[:, sl], kp[:, 0:gc * NT])
                    else:
                        nc.vector.tensor_copy(dst[:, sl], kp[:, 0:gc * NT])

            # ---- vT1: [128, 32 j-tiles, 33] tiles, ones in col 32 ----
            vT1 = cp.tile([128, NJT, 33], BF16, tag="vT1")
            for half in range(2):
                vp = ps_o.tile([128, NT], F32, tag="o")
                for jj in range(16):
                    jt = 16 * half + jj
                    nc.tensor.matmul(
                        vp[:, DH * jj:DH * (jj + 1)],
                        x_sb[:, jt * JT:(jt + 1) * JT],
                        wv_sb[:],
                        start=True, stop=True)
                nc.vector.tensor_copy(
                    vT1[:, 16 * half:16 * (half + 1), 0:DH], vp[:])
            nc.vector.memset(vT1[:, :, DH], 1.0)

            # ---- attention over i-tiles ----
            # epilogue(t) is emitted after tile t+1's first group so the PE
            # stream reaches tile t+1's sim before stalling on tile t's evac.
            pending_epilogue = [None]

            for it in range(NIT):
                si = slice(it * NT, (it + 1) * NT)

                o_ps = ps_o.tile([128, NT], F32, tag="o")
                jbase = 0
                for g, gs in enumerate(GROUPS):
                    pp, tg, wd = (ps4, "s4", 2048) if gs == 4 else (ps3, "s3", 1536)
                    s_ps = pp.tile([128, wd], F32, tag=tg)
                    for m in range(gs):
                        j = jbase + m
                        nc.tensor.matmul(
                            s_ps[:, NT * m:NT * (m + 1)],
                            k4[32 * m:32 * m + 32, j * JT:(j + 1) * JT],
                            q4[32 * m:32 * m + 32, si],
                            start=True, stop=True,
                            tile_position=(32 * m, 0))
                    pT = wp3.tile([128, 2048], BF16, tag="pT")
                    if EXP_DVE[g]:
                        nc.vector.tensor_scalar(
                            pT[:, 0:NT * gs].bitcast(I16),
                            s_ps[:, 0:NT * gs], EXPA, EXPC, MULT, ADD)
                    else:
                        nc.scalar.activation(pT[:, 0:NT * gs],
                                             s_ps[:, 0:NT * gs], AF.Exp)
                    for m in range(gs):
                        j = jbase + m
                        side = j % 2
                        nc.tensor.matmul(
                            o_ps[0:33, :] if side == 0 else o_ps[64:97, :],
                            vT1[:, j, :],
                            pT[:, NT * m:NT * (m + 1)],
                            start=(j < 2), stop=(j >= NJT - 2),
                            tile_position=(0, 0) if side == 0 else (0, 64),
                            skip_group_check=True)
                    jbase += gs
                    if g == 1 and pending_epilogue[0] is not None:
                        pending_epilogue[0]()
                        pending_epilogue[0] = None

                def make_epilogue(o_ps=o_ps, si=si):
                    def epi():
                        o_sb = wp.tile([98, NT], F32, tag="o_sb")
                        nc.vector.tensor_copy(o_sb[:], o_ps[0:98, :])
                        nc.sync.dma_start(o_out.ap()[:, si], o_sb[:])
                    return epi

                pending_epilogue[0] = make_epilogue()

            pending_epilogue[0]()

    nc.compile()
    _CACHE["nc"] = nc
    return nc


def make_in_maps(x, w_qkv, w_out, b_out):
    x = np.asarray(x, dtype=np.float32)
    w_qkv = np.asarray(w_qkv, dtype=np.float32)

    xf = np.ascontiguousarray(x.reshape(B, C, N))
    wq = w_qkv[0:C].reshape(HEADS, DH, C)
    wk = w_qkv[C:2 * C].reshape(HEADS, DH, C)
    wv = w_qkv[2 * C:3 * C].reshape(HEADS, DH, C)

    in_maps = []
    for core in range(8):
        b_i, h_i = divmod(core, HEADS)
        in_maps.append({
            "xt": xf[b_i],
            "wq4": np.ascontiguousarray(np.tile((wq[h_i] * SCALE).T, (1, 4))),
            "wk4": np.ascontiguousarray(np.tile(wk[h_i].T, (1, 4))),
            "wv": np.ascontiguousarray(wv[h_i].T),
        })
    return in_maps


def kernel(x, w_qkv, w_out, b_out):
    from concourse.bass_utils import run_bass_kernel_spmd

    x = np.asarray(x, dtype=np.float32)
    w_out = np.asarray(w_out, dtype=np.float32)
    b_out = np.asarray(b_out, dtype=np.float32)
    xf = np.ascontiguousarray(x.reshape(B, C, N))

    in_maps = make_in_maps(x, w_qkv, w_out, b_out)

    nc = _build()
    res = run_bass_kernel_spmd(nc, in_maps, core_ids=list(range(8)))

    # host epilogue: normalize, output-project, sum heads, bias + residual
    outf = np.tile(b_out[None, :, None], (B, 1, N)) + xf
    for core in range(8):
        b_i, h_i = divmod(core, HEADS)
        o = res.results[core]["o_out"]
        o33 = o[0:33] + o[64:97]                       # merge col-tile halves
        attn = o33[0:DH] / o33[DH][None, :]            # normalize
        woh = w_out[:, h_i * DH:(h_i + 1) * DH]        # [C, DH]
        outf[b_i] += woh @ attn
    return outf.reshape(B, C, H, W).astype(np.float32)


# revision 29
# speedup vs baseline: 2.3704x; 2.3704x over previous
"""Trainium2 Bass kernel for spatial multi-head self-attention
(conv1x1 qkv -> 4-head attention over n=4096 tokens -> conv1x1 out + residual).

Sharding: 8 cores = 2 batches x 4 heads; each core runs one (batch, head)
attention and emits the UN-normalized head context [V^T|1]P (33 rows: 32 dims
+ softmax denominator) for its head; the host divides by the denominator,
applies the output projection, sums the 4 head partials per batch and adds
bias + residual (small numpy epilogue).

Device pipeline per core (all matmuls bf16/f32r, fp32 accumulate):
  prep: x DMA'd f32 and used directly as f32r (bitcast, no convert);
        k4/q4 = head K/Q replicated at partition blocks 0/32/64/96 so sim
        matmuls row-pack 4x (K=32, concurrent row-tiles); vT1[j] = [V^T|1].
  per i-tile (512 tokens): 9 sim groups of 4/3 j-tiles (row-packed, PSUM
        4-bank/3-bank alternating) -> exp SPLIT across two engines:
        ACT does native Exp on the 4-groups, DVE does Schraudolph exp
        (tensor_scalar f32*A+C -> int16 -> bitcast bf16) on the 3-groups --
        the exp wall is the kernel's roofline and the split nearly doubles
        exp throughput; AV matmuls are 2x column-tiled (even j-tiles ->
        PSUM partitions 0:33, odd -> 64:97, concurrent col groups).
  epilogue per i-tile: one DVE copy PSUM->SBUF + DMA out (no on-device
        normalize/projection).
"""

import numpy as np

B, C, H, W = 2, 128, 64, 64
N = H * W            # 4096
HEADS = 4
DH = 32              # head dim
NT = 512             # i-tile width
NIT = N // NT        # 8 i-tiles
JT = 128             # j-tile width
NJT = N // JT        # 32 j-tiles
GROUPS = [3, 3, 3, 3, 3, 3, 3, 3, 3, 3, 2]    # j-tiles per sim/exp group
EXP_DVE = [False] * 11
SCALE = DH ** -0.5
EXPA = 128.0 / np.log(2.0)   # Schraudolph bf16: bits = rint(s*EXPA + EXPC)
EXPC = 16248.6

_CACHE = {}


def _build():
    if "nc" in _CACHE:
        return _CACHE["nc"]

    import concourse.bacc as bacc
    import concourse.mybir as mybir
    import concourse.tile as tile

    F32 = mybir.dt.float32
    F32R = mybir.dt.float32r
    BF16 = mybir.dt.bfloat16
    I16 = mybir.dt.int16
    AF = mybir.ActivationFunctionType
    MULT = mybir.AluOpType.mult
    ADD = mybir.AluOpType.add

    nc = bacc.Bacc("TRN2", target_bir_lowering=False, debug=False, num_devices=8)

    xt = nc.dram_tensor("xt", [C, N], F32R, kind="ExternalInput")
    wq4 = nc.dram_tensor("wq4", [C, 128], F32R, kind="ExternalInput")
    wk4 = nc.dram_tensor("wk4", [C, 128], F32R, kind="ExternalInput")
    wv = nc.dram_tensor("wv", [C, DH], F32R, kind="ExternalInput")
    o_out = nc.dram_tensor("o_out", [33, N], F32, kind="ExternalOutput")

    with tile.TileContext(nc) as tc:
        with (
            tc.tile_pool(name="const", bufs=1) as cp,
            tc.tile_pool(name="work", bufs=2) as wp,
            tc.tile_pool(name="work3", bufs=3) as wp3,
            tc.tile_pool(name="work3d", bufs=3) as wp3d,
            tc.tile_pool(name="ps_sim4", bufs=1, space="PSUM") as ps4,
            tc.tile_pool(name="ps_sim3", bufs=1, space="PSUM") as ps3,
            tc.tile_pool(name="ps_o", bufs=1, space="PSUM") as ps_o,
        ):
            # ---- weights (f32 staged, viewed as f32r for the PE) ----
            wq_sb = cp.tile([C, 128], F32R, tag="wq_sb")
            nc.sync.dma_start(wq_sb[:], wq4.ap())
            wk_sb = cp.tile([C, 128], F32R, tag="wk_sb")
            nc.sync.dma_start(wk_sb[:], wk4.ap())
            wv_sb = cp.tile([C, DH], F32R, tag="wv_sb")
            nc.sync.dma_start(wv_sb[:], wv.ap())

            # ---- x: chunked DMA, no conversion (f32r view) ----
            x_sb = cp.tile([C, N], F32R, tag="x_sb")
            for ci in range(NIT):
                s = slice(ci * NT, (ci + 1) * NT)
                nc.sync.dma_start(x_sb[:, s], xt.ap()[:, s])

            # ---- k4 / q4 projections (replica-packed stationary) ----
            # chunks of 3/3/2 through the 4-bank / 3-bank sim pools so the
            # evacuations pipeline; evac alternates ACT/DVE.
            k4 = cp.tile([128, N], BF16, tag="k4")
            q4 = cp.tile([128, N], BF16, tag="q4")
            for dst, wsb in ((k4, wk_sb), (q4, wq_sb)):
                for gi, (c0, gc) in enumerate(((0, 3), (3, 3), (6, 2))):
                    pp = ps4 if gi % 2 == 0 else ps3
                    kp = pp.tile([128, 1536], F32,
                                 tag="s4" if gi % 2 == 0 else "s3")
                    for m in range(gc):
                        ch = c0 + m
                        nc.tensor.matmul(
                            kp[:, NT * m:NT * (m + 1)],
                            wsb[:],
                            x_sb[:, ch * NT:(ch + 1) * NT],
                            start=True, stop=True)
                    sl = slice(c0 * NT, (c0 + gc) * NT)
                    if gi % 2 == 0:
                        nc.scalar.copy(dst[:, sl], kp[:, 0:gc * NT])
                    else:
                        nc.vector.tensor_copy(dst[:, sl], kp[:, 0:gc * NT])

            # ---- vT1: [128, 32 j-tiles, 33] tiles, ones in col 32 ----
            vT1 = cp.tile([128, NJT, 33], BF16, tag="vT1")
            for half in range(2):
                vp = ps_o.tile([128, NT], F32, tag="o")
                for jj in range(16):
                    jt = 16 * half + jj
                    nc.tensor.matmul(
                        vp[:, DH * jj:DH * (jj + 1)],
                        x_sb[:, jt * JT:(jt + 1) * JT],
                        wv_sb[:],
                        start=True, stop=True)
                nc.vector.tensor_copy(
                    vT1[:, 16 * half:16 * (half + 1), 0:DH], vp[:])
            nc.vector.memset(vT1[:, :, DH], 1.0)

            # ---- attention over i-tiles (software-pipelined emission) ----
            # PE is strictly in-order, so AV(g) [which waits on exp(g)] must
            # not sit in front of sim(g+1) in the PE queue: emit sim two
            # groups ahead of the exp->AV pair. With exp alternating between
            # ACT and DVE, the two exp streams then run concurrently.
            descs = []
            for it in range(NIT):
                jbase = 0
                for g, gs in enumerate(GROUPS):
                    descs.append((it, g, jbase, gs))
                    jbase += gs
            s_handles = {}
            o_handles = {}

            def emit_sim(k):
                it, g, jbase, gs = descs[k]
                si = slice(it * NT, (it + 1) * NT)
                pp, tg = (ps4, "s4") if k % 2 == 0 else (ps3, "s3")
                s_ps = pp.tile([128, 1536], F32, tag=tg, name=f"s{k}")
                for m in range(gs):
                    j = jbase + m
                    nc.tensor.matmul(
                        s_ps[:, NT * m:NT * (m + 1)],
                        k4[32 * m:32 * m + 32, j * JT:(j + 1) * JT],
                        q4[32 * m:32 * m + 32, si],
                        start=True, stop=True,
                        tile_position=(32 * m, 0))
                s_handles[k] = s_ps

            def emit_epilogue(it):
                o_sb = wp.tile([33, NT], F32, tag="o_sb", name=f"ob{it}")
                nc.vector.tensor_copy(o_sb[:], o_handles[it][0:33, :])
                nc.sync.dma_start(
                    o_out.ap()[:, it * NT:(it + 1) * NT], o_sb[:])

            emit_sim(0)
            emit_sim(1)
            for k in range(len(descs)):
                it, g, jbase, gs = descs[k]
                s_ps = s_handles.pop(k)
                pT = wp3.tile([128, 2048], BF16, tag="pT", name=f"p{k}")
                wa = (NT * gs * 45 // 64) // 64 * 64
                nc.scalar.activation(pT[:, 0:wa], s_ps[:, 0:wa], AF.Exp)
                nc.vector.tensor_scalar(
                    pT[:, wa:NT * gs].bitcast(I16),
                    s_ps[:, wa:NT * gs], EXPA, EXPC, MULT, ADD)
                if g == 0:
                    if it > 0:
                        emit_epilogue(it - 1)
                    o_handles[it] = ps_o.tile([128, NT], F32, tag="o",
                                              name=f"o{it}")
                o_ps = o_handles[it]
                for m in range(gs):
                    j = jbase + m
                    nc.tensor.matmul(
                        o_ps[0:33, :],
                        vT1[:, j, :],
                        pT[:, NT * m:NT * (m + 1)],
                        start=(j == 0), stop=(j == NJT - 1),
                        skip_group_check=True)
                if k + 2 < len(descs):
                    emit_sim(k + 2)
            emit_epilogue(NIT - 1)

    nc.compile()
    _CACHE["nc"] = nc
    return nc


def make_in_maps(x, w_qkv, w_out, b_out):
    x = np.asarray(x, dtype=np.float32)
    w_qkv = np.asarray(w_qkv, dtype=np.float32)

    xf = np.ascontiguousarray(x.reshape(B, C, N))
    wq = w_qkv[0:C].reshape(HEADS, DH, C)
    wk = w_qkv[C:2 * C].reshape(HEADS, DH, C)
    wv = w_qkv[2 * C:3 * C].reshape(HEADS, DH, C)

    in_maps = []
    for core in range(8):
        b_i, h_i = divmod(core, HEADS)
        in_maps.append({
            "xt": xf[b_i],
            "wq4": np.ascontiguousarray(np.tile((wq[h_i] * SCALE).T, (1, 4))),
            "wk4": np.ascontiguousarray(np.tile(wk[h_i].T, (1, 4))),
            "wv": np.ascontiguousarray(wv[h_i].T),
        })
    return in_maps


def kernel(x, w_qkv, w_out, b_out):
    from concourse.bass_utils import run_bass_kernel_spmd

    x = np.asarray(x, dtype=np.float32)
    w_out = np.asarray(w_out, dtype=np.float32)
    b_out = np.asarray(b_out, dtype=np.float32)
    xf = np.ascontiguousarray(x.reshape(B, C, N))

    in_maps = make_in_maps(x, w_qkv, w_out, b_out)

    nc = _build()
    res = run_bass_kernel_spmd(nc, in_maps, core_ids=list(range(8)))

    # host epilogue: normalize, output-project, sum heads, bias + residual
    outf = np.tile(b_out[None, :, None], (B, 1, N)) + xf
    for core in range(8):
        b_i, h_i = divmod(core, HEADS)
        o33 = res.results[core]["o_out"]
        attn = o33[0:DH] / o33[DH][None, :]            # normalize
        woh = w_out[:, h_i * DH:(h_i + 1) * DH]        # [C, DH]
        outf[b_i] += woh @ attn
    return outf.reshape(B, C, H, W).astype(np.float32)
